# revision 1
# baseline (speedup 1.0000x reference)
"""AttentionPooling TRN2 kernel: 8-core data-parallel over flattened (B*N) points.

Math (per point n with k=16 neighbors, C=512 channels):
  logits = x @ w_score.T            (per-channel attention logits)
  scores = softmax_k(logits)        (softmax over the k axis, per channel)
  pooled = sum_k x * scores
  y      = relu((pooled @ w_conv.T - mean) * gamma/sqrt(var+eps) + beta)

Device mapping (per core, 2048 n-points = 32768 (n,k) rows):
  - x rows (pt=(n,k) on partitions, c on free) feed the elementwise product.
  - mm1 uses bf16 copies of x loaded TRANSPOSED via the DMA xbar (c on
    partitions) as the stationary operand: logits = xT.T @ w_score.T.
  - softmax-over-k reductions run on the TensorEngine as matmuls with a
    0/1 group matrix G (k groups live in partition dim), accumulating 16
    chunks into one packed (128 n, 512 c) PSUM tile.
  - BN is folded into w_conv (scale) + a rank-1 bias matmul; ReLU on ScalarE.
  - float32r (fp32 with 11-bit mantissa, 4x matmul throughput) is used for
    everything except the bf16 mm1.  float32r tiles are only ever produced
    by compute engines (ACT/DVE) -- f32r-typed DMAs apply rounding in the
    DMA datapath and corrupt concurrent xbar-transpose DMAs.
"""
import numpy as np
import ml_dtypes

B, N, K, C, COUT = 4, 4096, 16, 512, 512
NCORES = 8
PTS_PER_CORE = B * N * K // NCORES      # 32768
NROWS_PER_CORE = B * N // NCORES        # 2048 n-points
NSB = NROWS_PER_CORE // 128             # 16 super-blocks of 128 n
NCHUNK = 16                             # chunks of 128 (n,k) rows per super-block
BN_EPS = 1e-5

_cached = {}


def _build():
    import concourse.bacc as bacc
    import concourse.mybir as mybir
    import concourse.tile as tile

    F32, F32R, BF16 = mybir.dt.float32, mybir.dt.float32r, mybir.dt.bfloat16
    ACT = mybir.ActivationFunctionType

    nc = bacc.Bacc("TRN2", target_bir_lowering=False, debug=False, num_devices=NCORES)
    xf = nc.dram_tensor("xf", [PTS_PER_CORE, C], F32, kind="ExternalInput")
    xb = nc.dram_tensor("xb", [PTS_PER_CORE, C], BF16, kind="ExternalInput")
    wst = nc.dram_tensor("wst", [C, C], BF16, kind="ExternalInput")
    wc2t = nc.dram_tensor("wc2t", [C, COUT], F32, kind="ExternalInput")
    bias2 = nc.dram_tensor("bias2", [1, COUT], F32, kind="ExternalInput")
    ones = nc.dram_tensor("ones", [1, 128], F32, kind="ExternalInput")
    gmat = nc.dram_tensor("gmat", [128, 128 * NCHUNK], F32, kind="ExternalInput")
    ident = nc.dram_tensor("ident", [128, 128], F32, kind="ExternalInput")
    y = nc.dram_tensor("y", [NROWS_PER_CORE, COUT], F32, kind="ExternalOutput")

    with tile.TileContext(nc) as tc:
        with (
            tc.tile_pool(name="const", bufs=1) as cp,
            tc.tile_pool(name="xT", bufs=2) as xtp,
            tc.tile_pool(name="work", bufs=3) as wp,
            tc.tile_pool(name="tail", bufs=2) as tp,
            tc.tile_pool(name="pl", bufs=2, space="PSUM") as pslp,
            tc.tile_pool(name="psacc", bufs=2, space="PSUM") as psa,
            tc.tile_pool(name="pstail", bufs=1, space="PSUM") as pst,
        ):
            # ---- constants (f32r ones produced via DVE copy, never DMA) ----
            wst_t = [cp.tile([128, C], BF16, tag=f"wst{i}", name=f"wst{i}") for i in range(4)]
            for i in range(4):
                nc.sync.dma_start(wst_t[i][:], wst[128 * i:128 * (i + 1), :])
            wc2t_f = [cp.tile([128, COUT], F32, tag=f"wc2tf{i}", name=f"wc2tf{i}") for i in range(4)]
            wc2t_t = [cp.tile([128, COUT], F32R, tag=f"wc2t{i}", name=f"wc2t{i}") for i in range(4)]
            for i in range(4):
                nc.sync.dma_start(wc2t_f[i][:], wc2t[128 * i:128 * (i + 1), :])
                nc.vector.tensor_copy(wc2t_t[i][:], wc2t_f[i][:])
            bias2_f = cp.tile([1, COUT], F32, tag="bias2f")
            nc.sync.dma_start(bias2_f[:], bias2[:])
            bias2_t = cp.tile([1, COUT], F32R, tag="bias2")
            nc.vector.tensor_copy(bias2_t[:], bias2_f[:])
            ones_f = cp.tile([1, 128], F32, tag="onesf")
            nc.sync.dma_start(ones_f[:], ones[:])
            ones_t = cp.tile([1, 128], F32R, tag="ones")
            nc.vector.tensor_copy(ones_t[:], ones_f[:])
            g_f = cp.tile([128, 128 * NCHUNK], F32, tag="gf")
            nc.sync.dma_start(g_f[:], gmat[:])
            g_t = cp.tile([128, 128 * NCHUNK], F32R, tag="g")
            nc.vector.tensor_copy(g_t[:], g_f[:])
            id_f = cp.tile([128, 128], F32, tag="identf")
            nc.sync.dma_start(id_f[:], ident[:])
            id_t = cp.tile([128, 128], F32R, tag="ident")
            nc.vector.tensor_copy(id_t[:], id_f[:])

            for sb in range(NSB):
                row0 = 2048 * sb
                xT = [
                    xtp.tile([128, 2048], BF16, tag=f"xT{i}", name=f"xT{i}_{sb}")
                    for i in range(4)
                ]
                for i in range(4):
                    nc.sync.dma_start(
                        xT[i][:],
                        xb[row0:row0 + 2048, 128 * i:128 * (i + 1)],
                        transpose=True,
                    )
                pe_sum = psa.tile([128, C], F32, tag="esum", name=f"esum{sb}")
                pt_sum = psa.tile([128, C], F32, tag="tsum", name=f"tsum{sb}")
                for j in range(NCHUNK):
                    xj = wp.tile([128, C], F32, tag="xj", name=f"xj{sb}_{j}")
                    nc.sync.dma_start(xj[:], xf[row0 + 128 * j:row0 + 128 * (j + 1), :])
                    pl = pslp.tile([128, C], F32, tag="pl", name=f"pl{sb}_{j}")
                    for i in range(4):
                        nc.tensor.matmul(
                            pl[:],
                            xT[i][:, 128 * j:128 * (j + 1)],
                            wst_t[i][:],
                            start=(i == 0),
                            stop=(i == 3),
                        )
                    ej = wp.tile([128, C], F32R, tag="ej", name=f"ej{sb}_{j}")
                    nc.scalar.activation(ej[:], pl[:], ACT.Exp)
                    tj = wp.tile([128, C], F32R, tag="tj", name=f"tj{sb}_{j}")
                    nc.vector.tensor_mul(tj[:], xj[:], ej[:])
                    gj = g_t[:, 128 * j:128 * (j + 1)]
                    nc.tensor.matmul(
                        pe_sum[:], gj, ej[:], start=(j == 0), stop=(j == NCHUNK - 1)
                    )
                    nc.tensor.matmul(
                        pt_sum[:], gj, tj[:], start=(j == 0), stop=(j == NCHUNK - 1)
                    )

                inv_t = tp.tile([128, C], F32, tag="inv", name=f"inv{sb}")
                nc.vector.reciprocal(inv_t[:], pe_sum[:])
                pooled = tp.tile([128, C], F32R, tag="pooled", name=f"pooled{sb}")
                nc.vector.tensor_mul(pooled[:], pt_sum[:], inv_t[:])

                ppT = pst.tile([128, C], F32R, tag="ppT", name=f"ppT{sb}")
                for i in range(4):
                    nc.tensor.transpose(
                        ppT[:, 128 * i:128 * (i + 1)],
                        pooled[:, 128 * i:128 * (i + 1)],
                        id_t[:],
                    )
                pT = tp.tile([128, C], F32R, tag="pT", name=f"pT{sb}")
                nc.scalar.copy(pT[:], ppT[:])

                py = pst.tile([128, COUT], F32, tag="py", name=f"py{sb}")
                for i in range(4):
                    nc.tensor.matmul(
                        py[:],
                        pT[:, 128 * i:128 * (i + 1)],
                        wc2t_t[i][:],
                        start=(i == 0),
                        stop=False,
                    )
                nc.tensor.matmul(py[:], ones_t[:], bias2_t[:], start=False, stop=True)
                y_t = tp.tile([128, COUT], F32, tag="yt", name=f"yt{sb}")
                nc.scalar.activation(y_t[:], py[:], ACT.Relu)
                nc.sync.dma_start(y[128 * sb:128 * (sb + 1), :], y_t[:])
    nc.compile()
    return nc


def _get_nc():
    if "nc" not in _cached:
        _cached["nc"] = _build()
    return _cached["nc"]


def _host_prep(x, w_score, w_conv, bn_gamma, bn_beta, bn_mean, bn_var):
    x = np.ascontiguousarray(np.asarray(x, dtype=np.float32)).reshape(B * N * K, C)
    w_score = np.asarray(w_score, dtype=np.float32)
    w_conv = np.asarray(w_conv, dtype=np.float32)
    inv = np.asarray(bn_gamma, dtype=np.float64) / np.sqrt(
        np.asarray(bn_var, dtype=np.float64) + BN_EPS
    )
    wc2 = w_conv.astype(np.float64) * inv[:, None]
    bias2 = (
        np.asarray(bn_beta, dtype=np.float64)
        - np.asarray(bn_mean, dtype=np.float64) * inv
    )
    g = np.zeros((128, 128 * NCHUNK), dtype=np.float32)
    for j in range(NCHUNK):
        for p in range(128):
            g[p, 128 * j + 8 * j + p // 16] = 1.0
    common = {
        "wst": np.ascontiguousarray(w_score.T).astype(ml_dtypes.bfloat16),
        "wc2t": np.ascontiguousarray(wc2.T).astype(np.float32),
        "bias2": bias2.reshape(1, COUT).astype(np.float32),
        "ones": np.ones((1, 128), dtype=np.float32),
        "gmat": g,
        "ident": np.eye(128, dtype=np.float32),
    }
    xb = x.astype(ml_dtypes.bfloat16)
    in_maps = []
    for c in range(NCORES):
        sl = slice(PTS_PER_CORE * c, PTS_PER_CORE * (c + 1))
        in_maps.append({"xf": x[sl], "xb": xb[sl], **common})
    return in_maps


def kernel(x, w_score, w_conv, bn_gamma, bn_beta, bn_mean, bn_var):
    from concourse.bass_utils import run_bass_kernel_spmd

    nc = _get_nc()
    in_maps = _host_prep(x, w_score, w_conv, bn_gamma, bn_beta, bn_mean, bn_var)
    res = run_bass_kernel_spmd(nc, in_maps, core_ids=list(range(NCORES)))
    out = np.concatenate([res.results[c]["y"] for c in range(NCORES)], axis=0)
    return out.reshape(B, N, COUT).astype(np.float32)


# revision 3
# speedup vs baseline: 1.2313x; 1.2313x over previous
"""AttentionPooling TRN2 kernel: 8-core data-parallel over flattened (B*N) points.

Math (per point n with k=16 neighbors, C=512 channels):
  logits = x @ w_score.T            (per-channel attention logits)
  scores = softmax_k(logits)        (softmax over the k axis, per channel)
  pooled = sum_k x * scores
  y      = relu((pooled @ w_conv.T - mean) * gamma/sqrt(var+eps) + beta)

Device mapping (per core, 2048 n-points = 32768 (n,k) rows):
  - x rows (pt=(n,k) on partitions, c on free) feed the elementwise product.
  - mm1 uses bf16 copies of x loaded TRANSPOSED via the DMA xbar (c on
    partitions) as the stationary operand: logits = xT.T @ w_score.T.
  - softmax-over-k reductions run on the TensorEngine as matmuls with a
    0/1 group matrix G (k groups live in partition dim), accumulating 16
    chunks into one packed (128 n, 512 c) PSUM tile.
  - BN is folded into w_conv (scale) + a rank-1 bias matmul; ReLU on ScalarE.
  - float32r (fp32 with 11-bit mantissa, 4x matmul throughput) is used for
    everything except the bf16 mm1.  float32r tiles are only ever produced
    by compute engines (ACT/DVE) -- f32r-typed DMAs apply rounding in the
    DMA datapath and corrupt concurrent xbar-transpose DMAs.
"""
import numpy as np
import ml_dtypes

B, N, K, C, COUT = 4, 4096, 16, 512, 512
NCORES = 8
PTS_PER_CORE = B * N * K // NCORES      # 32768
NROWS_PER_CORE = B * N // NCORES        # 2048 n-points
NSB = NROWS_PER_CORE // 128             # 16 super-blocks of 128 n
NCHUNK = 16                             # chunks of 128 (n,k) rows per super-block
BN_EPS = 1e-5

_cached = {}


def _build():
    import concourse.bacc as bacc
    import concourse.mybir as mybir
    import concourse.tile as tile

    F32, F32R, BF16 = mybir.dt.float32, mybir.dt.float32r, mybir.dt.bfloat16
    ACT = mybir.ActivationFunctionType

    nc = bacc.Bacc("TRN2", target_bir_lowering=False, debug=False, num_devices=NCORES)
    xf = nc.dram_tensor("xf", [PTS_PER_CORE, C], F32, kind="ExternalInput")
    xb = nc.dram_tensor("xb", [PTS_PER_CORE, C], BF16, kind="ExternalInput")
    wst = nc.dram_tensor("wst", [C, C], BF16, kind="ExternalInput")
    wc2t = nc.dram_tensor("wc2t", [C, COUT], F32, kind="ExternalInput")
    bias2 = nc.dram_tensor("bias2", [1, COUT], F32, kind="ExternalInput")
    ones = nc.dram_tensor("ones", [1, 128], F32, kind="ExternalInput")
    gmat = nc.dram_tensor("gmat", [128, 128 * NCHUNK], F32, kind="ExternalInput")
    ident = nc.dram_tensor("ident", [128, 128], F32, kind="ExternalInput")
    y = nc.dram_tensor("y", [NROWS_PER_CORE, COUT], F32, kind="ExternalOutput")

    with tile.TileContext(nc) as tc:
        with (
            tc.tile_pool(name="const", bufs=1) as cp,
            tc.tile_pool(name="xT", bufs=2) as xtp,
            tc.tile_pool(name="work", bufs=4) as wp,
            tc.tile_pool(name="tail", bufs=2) as tp,
            tc.tile_pool(name="pl", bufs=3, space="PSUM") as pslp,
            tc.tile_pool(name="psacc", bufs=2, space="PSUM") as psa,
            tc.tile_pool(name="pstail", bufs=1, space="PSUM") as pst,
        ):
            # ---- constants (f32r ones produced via DVE copy, never DMA) ----
            wst_t = [cp.tile([128, C], BF16, tag=f"wst{i}", name=f"wst{i}") for i in range(4)]
            for i in range(4):
                nc.sync.dma_start(wst_t[i][:], wst[128 * i:128 * (i + 1), :])
            wc2t_f = [cp.tile([128, COUT], F32, tag=f"wc2tf{i}", name=f"wc2tf{i}") for i in range(4)]
            wc2t_t = [cp.tile([128, COUT], F32R, tag=f"wc2t{i}", name=f"wc2t{i}") for i in range(4)]
            for i in range(4):
                nc.sync.dma_start(wc2t_f[i][:], wc2t[128 * i:128 * (i + 1), :])
                nc.vector.tensor_copy(wc2t_t[i][:], wc2t_f[i][:])
            bias2_f = cp.tile([1, COUT], F32, tag="bias2f")
            nc.sync.dma_start(bias2_f[:], bias2[:])
            bias2_t = cp.tile([1, COUT], F32R, tag="bias2")
            nc.vector.tensor_copy(bias2_t[:], bias2_f[:])
            ones_f = cp.tile([1, 128], F32, tag="onesf")
            nc.sync.dma_start(ones_f[:], ones[:])
            ones_t = cp.tile([1, 128], F32R, tag="ones")
            nc.vector.tensor_copy(ones_t[:], ones_f[:])
            g_f = cp.tile([128, 128 * NCHUNK], F32, tag="gf")
            nc.sync.dma_start(g_f[:], gmat[:])
            g_t = cp.tile([128, 128 * NCHUNK], F32R, tag="g")
            nc.vector.tensor_copy(g_t[:], g_f[:])
            id_f = cp.tile([128, 128], F32, tag="identf")
            nc.sync.dma_start(id_f[:], ident[:])
            id_t = cp.tile([128, 128], F32R, tag="ident")
            nc.vector.tensor_copy(id_t[:], id_f[:])

            # Software-pipelined chunk loop over all NSB*NCHUNK chunks.
            # The reduction (G) matmuls for chunk c are emitted GDELAY chunks
            # late so the PE never waits on the exp/mul chain of the chunk it
            # just produced; super-block tails are emitted right after that
            # block's last G-matmul, which lands a couple of chunks into the
            # next super-block's mm1 stream.
            GDELAY = 2
            TOTAL = NSB * NCHUNK
            xT_all = {}
            chunk_et = {}
            acc = {}

            def issue_transposes(sb):
                xT = [
                    xtp.tile([128, 2048], BF16, tag=f"xT{i}", name=f"xT{i}_{sb}")
                    for i in range(4)
                ]
                row0 = 2048 * sb
                for i in range(4):
                    nc.sync.dma_start(
                        xT[i][:],
                        xb[row0:row0 + 2048, 128 * i:128 * (i + 1)],
                        transpose=True,
                    )
                xT_all[sb] = xT

            def issue_tail(sb):
                pe_sum, pt_sum = acc.pop(sb)
                inv_t = tp.tile([128, C], F32, tag="inv", name=f"inv{sb}")
                nc.vector.reciprocal(inv_t[:], pe_sum[:])
                pooled = tp.tile([128, C], F32R, tag="pooled", name=f"pooled{sb}")
                nc.vector.tensor_mul(pooled[:], pt_sum[:], inv_t[:])
                ppT = pst.tile([128, C], F32R, tag="pstail", name=f"ppT{sb}")
                for i in range(4):
                    nc.tensor.transpose(
                        ppT[:, 128 * i:128 * (i + 1)],
                        pooled[:, 128 * i:128 * (i + 1)],
                        id_t[:],
                    )
                pT = tp.tile([128, C], F32R, tag="pT", name=f"pT{sb}")
                nc.scalar.copy(pT[:], ppT[:])
                py = pst.tile([128, COUT], F32, tag="pstail", name=f"py{sb}")
                for i in range(4):
                    nc.tensor.matmul(
                        py[:],
                        pT[:, 128 * i:128 * (i + 1)],
                        wc2t_t[i][:],
                        start=(i == 0),
                        stop=False,
                    )
                nc.tensor.matmul(py[:], ones_t[:], bias2_t[:], start=False, stop=True)
                y_t = tp.tile([128, COUT], F32, tag="yt", name=f"yt{sb}")
                nc.scalar.activation(y_t[:], py[:], ACT.Relu)
                nc.sync.dma_start(y[128 * sb:128 * (sb + 1), :], y_t[:])

            issue_transposes(0)
            for c in range(TOTAL + GDELAY):
                sb, j = divmod(c, NCHUNK)
                if c < TOTAL:
                    if j == 1 and sb + 1 < NSB:
                        issue_transposes(sb + 1)
                    row0 = 2048 * sb
                    xT = xT_all[sb]
                    xj = wp.tile([128, C], F32, tag="xj", name=f"xj{c}")
                    nc.sync.dma_start(
                        xj[:], xf[row0 + 128 * j:row0 + 128 * (j + 1), :]
                    )
                    pl = pslp.tile([128, C], F32, tag="pl", name=f"pl{c}")
                    for i in range(4):
                        nc.tensor.matmul(
                            pl[:],
                            xT[i][:, 128 * j:128 * (j + 1)],
                            wst_t[i][:],
                            start=(i == 0),
                            stop=(i == 3),
                        )
                    ej = wp.tile([128, C], F32R, tag="ej", name=f"ej{c}")
                    nc.scalar.activation(ej[:], pl[:], ACT.Exp)
                    tj = wp.tile([128, C], F32R, tag="tj", name=f"tj{c}")
                    nc.vector.tensor_mul(tj[:], xj[:], ej[:])
                    chunk_et[c] = (ej, tj)
                d = c - GDELAY
                if 0 <= d < TOTAL:
                    dsb, dj = divmod(d, NCHUNK)
                    if dj == 0:
                        acc[dsb] = (
                            psa.tile([128, C], F32, tag="esum", name=f"esum{dsb}"),
                            psa.tile([128, C], F32, tag="tsum", name=f"tsum{dsb}"),
                        )
                    pe_sum, pt_sum = acc[dsb]
                    ej, tj = chunk_et.pop(d)
                    gj = g_t[:, 128 * dj:128 * (dj + 1)]
                    nc.tensor.matmul(
                        pe_sum[:], gj, ej[:], start=(dj == 0), stop=(dj == NCHUNK - 1)
                    )
                    nc.tensor.matmul(
                        pt_sum[:], gj, tj[:], start=(dj == 0), stop=(dj == NCHUNK - 1)
                    )
                    if dj == NCHUNK - 1:
                        issue_tail(dsb)
    nc.compile()
    return nc


def _get_nc():
    if "nc" not in _cached:
        _cached["nc"] = _build()
    return _cached["nc"]


def _host_prep(x, w_score, w_conv, bn_gamma, bn_beta, bn_mean, bn_var):
    x = np.ascontiguousarray(np.asarray(x, dtype=np.float32)).reshape(B * N * K, C)
    w_score = np.asarray(w_score, dtype=np.float32)
    w_conv = np.asarray(w_conv, dtype=np.float32)
    inv = np.asarray(bn_gamma, dtype=np.float64) / np.sqrt(
        np.asarray(bn_var, dtype=np.float64) + BN_EPS
    )
    wc2 = w_conv.astype(np.float64) * inv[:, None]
    bias2 = (
        np.asarray(bn_beta, dtype=np.float64)
        - np.asarray(bn_mean, dtype=np.float64) * inv
    )
    g = np.zeros((128, 128 * NCHUNK), dtype=np.float32)
    for j in range(NCHUNK):
        for p in range(128):
            g[p, 128 * j + 8 * j + p // 16] = 1.0
    common = {
        "wst": np.ascontiguousarray(w_score.T).astype(ml_dtypes.bfloat16),
        "wc2t": np.ascontiguousarray(wc2.T).astype(np.float32),
        "bias2": bias2.reshape(1, COUT).astype(np.float32),
        "ones": np.ones((1, 128), dtype=np.float32),
        "gmat": g,
        "ident": np.eye(128, dtype=np.float32),
    }
    xb = x.astype(ml_dtypes.bfloat16)
    in_maps = []
    for c in range(NCORES):
        sl = slice(PTS_PER_CORE * c, PTS_PER_CORE * (c + 1))
        in_maps.append({"xf": x[sl], "xb": xb[sl], **common})
    return in_maps


def kernel(x, w_score, w_conv, bn_gamma, bn_beta, bn_mean, bn_var):
    from concourse.bass_utils import run_bass_kernel_spmd

    nc = _get_nc()
    in_maps = _host_prep(x, w_score, w_conv, bn_gamma, bn_beta, bn_mean, bn_var)
    res = run_bass_kernel_spmd(nc, in_maps, core_ids=list(range(NCORES)))
    out = np.concatenate([res.results[c]["y"] for c in range(NCORES)], axis=0)
    return out.reshape(B, N, COUT).astype(np.float32)
